# revision 3
# baseline (speedup 1.0000x reference)
"""Trainium2 Bass kernel: CapOnlyContrastiveLoss (margin contrastive, mean).

loss = [sum_i EX*c_i - sum_ij min(sqrt(d2_ij), c_i)] / N^2
  d2_ij = ||im_i||^2 + ||ex_j||^2 - 2 im_i.ex_j,   c_i = margin + ||im_i - s_i||

Per core (4x2 grid: 2048 im rows x 4096 ex rows):
  * The O(N*D) row-vector preludes (||im_i||^2, c_i, ||ex_j||^2) are computed
    on the host during input sharding (~0.04% of the FLOPs; the O(N^2*D)
    score GEMM + epilogue run on device); they ship as tiny extra inputs.
  * fp8(e4m3) DoubleRow matmuls: psum = im.ex (K=512 as 2 DR matmuls of
    K=256) + one K=1 float32r matmul adding -exsq_j/2 per bank.
  * epilogue per [128 x 2048] span (4 PSUM banks): DVE computes
    m = min(-2*psum, c_i^2 - imsq_i) -> bf16 (plain tensor_scalar, two fused
    scalar ops), ACT computes sqrt(m + imsq_i) with its accumulator, giving
    acc_i += sum_j min(sqrt(d2_ij), c_i) in one ACT pass (no DVE reduce).
  * operand build: fp32 loads; fp32->bf16 casts split DVE/ACT; XBAR bf16
    transposes to k-major planes on two DMA queues; bf16->fp8 plane casts
    on DVE.
  * host finish: loss = sum_cores(EX_R * sum c - sum acc) / N^2.
"""

import numpy as np

import concourse.bacc as bacc
import concourse.bass as bass
import concourse.tile as tile
from concourse import bass_utils, mybir

N, D = 8192, 512
MARGIN = 0.2
P = 128
I_GROUPS, J_GROUPS = 4, 2
IM_R = N // I_GROUPS          # 2048
EX_R = N // J_GROUPS          # 4096
N_IT = IM_R // P              # 16
N_ET = EX_R // P              # 32
SPAN = 2048
NJ = 512
BANKS = SPAN // NJ            # 4
N_H = EX_R // SPAN            # 2
KC = D // P                   # 4

F32 = mybir.dt.float32
F32R = mybir.dt.float32r
BF16 = mybir.dt.bfloat16
F8 = mybir.dt.float8e4
AF = mybir.ActivationFunctionType
ALU = mybir.AluOpType
PM = mybir.MatmulPerfMode

_CACHE = {}


def _corr_rows(exsq_slice):
    import ml_dtypes
    v = (-0.5 * exsq_slice).astype(np.float32)
    hi = v.astype(ml_dtypes.bfloat16)
    lo = (v - hi.astype(np.float32)).astype(ml_dtypes.bfloat16)
    return np.ascontiguousarray(np.stack([hi, lo], axis=0))


def _emit(tc, nc, im_d, ex_d, imsq_d, cc_d, corr_d, acc_d):
    from contextlib import ExitStack

    with ExitStack() as ctx:
        singles = ctx.enter_context(tc.tile_pool(name="singles", bufs=1))
        ld_im = ctx.enter_context(tc.tile_pool(name="ld_im", bufs=2))
        ld_ex = ctx.enter_context(tc.tile_pool(name="ld_ex", bufs=4))
        stage = ctx.enter_context(tc.tile_pool(name="stage", bufs=8))
        sqp = ctx.enter_context(tc.tile_pool(name="sqp", bufs=8))
        psum = ctx.enter_context(tc.tile_pool(name="psum", bufs=2, space="PSUM"))

        imT8 = singles.tile([P, KC, IM_R], F8)
        exT8 = singles.tile([P, KC, EX_R], F8)
        ones2 = singles.tile([2, P], BF16)
        corr = singles.tile([2, EX_R], BF16)
        imsq = singles.tile([P, N_IT], F32)
        ccsqmi = singles.tile([P, N_IT], F32)
        acc_sb = singles.tile([P, N_H * N_IT], F32)
        junk = [singles.tile([P, SPAN], BF16, name=f"junk{i}")
                for i in range(2)]

        nc.vector.memset(ones2, 1.0)

        # tiny vector inputs first (instant, on scalar queue)
        nc.scalar.dma_start(out=imsq, in_=imsq_d)
        nc.scalar.dma_start(out=ccsqmi, in_=cc_d)
        nc.scalar.dma_start(out=corr, in_=corr_d)

        # big loads: 2MB each so every push gets its own DMA semaphore and
        # issues immediately (small DMAs stall the in-order queue on
        # completion-semaphore recycling, delaying the transposes queued
        # behind them)
        im_big, ex_big = [], []
        for q in range(4):
            te = ld_ex.tile([P, 8, D], F32, tag="ld_ex")
            nc.sync.dma_start(
                out=te, in_=ex_d[q * 1024:(q + 1) * 1024, :].rearrange(
                    "(c p) d -> p c d", p=P))
            ex_big.append(te)
        for g in range(2):
            ti = ld_im.tile([P, 8, D], F32, tag="ld_im")
            nc.scalar.dma_start(
                out=ti, in_=im_d[g * 1024:(g + 1) * 1024, :].rearrange(
                    "(c p) d -> p c d", p=P))
            im_big.append(ti)

        def im_t(t):
            return im_big[t // 8][:, t % 8, :]

        def ex_t(u):
            return ex_big[u // 8][:, u % 8, :]

        # ex chain first (it gates the most downstream work):
        # cast1 evens->ACT window, odds->Pool; XBAR on sync queue;
        # cast2 h0->DVE (fast, critical), h1->Pool (off critical path)
        for u in range(N_ET):
            exb = stage.tile([P, D], BF16, tag="exb")
            if u % 2 == 0:
                nc.scalar.copy(out=exb, in_=ex_t(u))
            else:
                nc.gpsimd.tensor_copy(out=exb, in_=ex_t(u))
            exTb = stage.tile([P, KC, P], BF16, tag="exTb")
            nc.sync.dma_start_transpose(exTb, exb)
            if u < N_ET // 2:
                nc.vector.tensor_copy(out=exT8[:, :, u * P:(u + 1) * P],
                                      in_=exTb)
            else:
                nc.gpsimd.tensor_copy(out=exT8[:, :, u * P:(u + 1) * P],
                                      in_=exTb)

        # im chain: DVE cast1 -> XBAR (scalar queue, after ex-c1 evens) ->
        # DVE cast2
        for t in range(N_IT):
            imb = stage.tile([P, D], BF16, tag="imb")
            nc.vector.tensor_copy(out=imb, in_=im_t(t))
            imTb = stage.tile([P, KC, P], BF16, tag="imTb")
            nc.scalar.dma_start_transpose(imTb, imb)
            nc.vector.tensor_copy(out=imT8[:, :, t * P:(t + 1) * P], in_=imTb)

        # ---- main loop ----
        def span(h, it):
            ps = psum.tile([P, SPAN], F32, tag="ps")
            for c in range(2):
                lhsT = imT8[:, 2 * c:2 * c + 2, it * P:(it + 1) * P]
                for b in range(BANKS):
                    js = h * SPAN + b * NJ
                    nc.tensor.matmul(ps[:, b * NJ:(b + 1) * NJ], lhsT,
                                     exT8[:, 2 * c:2 * c + 2, js:js + NJ],
                                     start=(c == 0), stop=False,
                                     perf_mode=PM.DoubleRow)
            for b in range(BANKS):
                js = h * SPAN + b * NJ
                nc.tensor.matmul(ps[:, b * NJ:(b + 1) * NJ], ones2,
                                 corr[:, js:js + NJ], start=False, stop=True)
            m = sqp.tile([P, SPAN], BF16, tag="m")
            nc.vector.tensor_scalar(m, ps, -2.0, ccsqmi[:, it:it + 1],
                                    ALU.mult, ALU.min)
            col = h * N_IT + it
            nc.scalar.activation(out=junk[col % 2], in_=m, func=AF.Sqrt,
                                 bias=imsq[:, it:it + 1],
                                 accum_out=acc_sb[:, col:col + 1])

        for h in range(N_H):
            for it in range(N_IT):
                span(h, it)

        nc.sync.dma_start(out=acc_d, in_=acc_sb)


def build_program():
    nc = bacc.Bacc("TRN2", target_bir_lowering=False, debug=False)
    im_d = nc.dram_tensor("im", [IM_R, D], F32, kind="ExternalInput").ap()
    ex_d = nc.dram_tensor("ex", [EX_R, D], F32, kind="ExternalInput").ap()
    imsq_d = nc.dram_tensor("imsq", [P, N_IT], F32, kind="ExternalInput").ap()
    cc_d = nc.dram_tensor("cc", [P, N_IT], F32, kind="ExternalInput").ap()
    corr_d = nc.dram_tensor("corr", [2, EX_R], BF16, kind="ExternalInput").ap()
    acc_d = nc.dram_tensor("acc", [P, N_H * N_IT], F32,
                           kind="ExternalOutput").ap()
    with tile.TileContext(nc) as tc:
        _emit(tc, nc, im_d, ex_d, imsq_d, cc_d, corr_d, acc_d)
    nc.compile()
    return nc


def get_program():
    if "nc" not in _CACHE:
        _CACHE["nc"] = build_program()
    return _CACHE["nc"]


def make_in_maps(im, s, ex_s):
    im = np.asarray(im, dtype=np.float32)
    s = np.asarray(s, dtype=np.float32)
    ex_s = np.asarray(ex_s, dtype=np.float32)
    imsq = np.einsum("ij,ij->i", im, im, dtype=np.float32)
    diff = im - s
    cc = (MARGIN + np.sqrt(np.einsum("ij,ij->i", diff, diff))).astype(np.float32)
    exsq = np.einsum("ij,ij->i", ex_s, ex_s, dtype=np.float32)
    ccsums = []
    in_maps = []
    for c in range(8):
        ig, jg = divmod(c, J_GROUPS)
        isl = slice(ig * IM_R, (ig + 1) * IM_R)
        jsl = slice(jg * EX_R, (jg + 1) * EX_R)
        in_maps.append({
            "im": np.ascontiguousarray(im[isl]),
            "ex": np.ascontiguousarray(ex_s[jsl]),
            # column layouts: value for row r lands at [r % 128, r // 128]
            "imsq": np.ascontiguousarray(imsq[isl].reshape(N_IT, P).T),
            "cc": np.ascontiguousarray(
                (cc[isl] ** 2 - imsq[isl]).reshape(N_IT, P).T),
            "corr": _corr_rows(exsq[jsl]),
        })
        ccsums.append(float(cc[isl].sum(dtype=np.float64)))
    return in_maps, ccsums


def finish(ccsums, results):
    total = 0.0
    for cs, r in zip(ccsums, results):
        total += float(EX_R) * cs
        total -= float(np.sum(r["acc"], dtype=np.float64))
    return np.array(total / (float(N) * float(N)), dtype=np.float32)


def kernel(im, s, ex_s):
    nc = get_program()
    in_maps, ccsums = make_in_maps(im, s, ex_s)
    res = bass_utils.run_bass_kernel_spmd(nc, in_maps, core_ids=list(range(8)))
    return finish(ccsums, res.results)


if __name__ == "__main__":
    rng = np.random.default_rng(0)
    im = rng.standard_normal((N, D), dtype=np.float32)
    s = rng.standard_normal((N, D), dtype=np.float32)
    ex = rng.standard_normal((N, D), dtype=np.float32)
    print(kernel(im, s, ex))
